# revision 26
# baseline (speedup 1.0000x reference)
"""Mixed causal attention (B=8,L=1024,D=1024,H=16,NS=8) on 8 TRN2 cores.

Sharding: data-parallel over batch (core b owns batch b) for projections,
attention, out-proj.  The per-position (ns) projection weights are sharded
by position: core c computes q/k/v for position 1016+c for ALL batches
(reads only Wq_ns[c],Wk_ns[c],Wv_ns[c]), then an AllGather distributes the
3x[8,1024] results; each core extracts its batch's 8 tail rows with a
one-hot selection matmul (the program is SPMD-identical, so per-core row
selection is driven by a per-core input, not program structure).

Attention layout: scores [k-part, q-free]; V tile carries a shared ones
column block per head pair so attn@V emits the numerator and a 64-way
replicated softmax denominator in one matmul; normalization is then one
fast approximate reciprocal + one multiply on the vector engine.
All score/exp/attn@V work is causally range-trimmed per k-block.
"""

import sys
import os
from contextlib import ExitStack

import numpy as np

sys.path.insert(0, "/opt/trn_rl_repo")

import ml_dtypes  # noqa: E402
import concourse.bass as bass  # noqa: E402
import concourse.tile as tile  # noqa: E402
from concourse import bacc, mybir  # noqa: E402
from concourse._compat import with_exitstack  # noqa: E402
from concourse.bass_utils import run_bass_kernel_spmd  # noqa: E402

B, L, D, H, NS = 8, 1024, 1024, 16, 8
HD = D // H          # 64
LS = L - NS          # 1016
NCORES = 8
NEG = -1.0e9
BF = mybir.dt.bfloat16
F32 = mybir.dt.float32

# vb layout: per lt block of 1536 cols, 8 head-pairs of 192 cols each:
# [v_{2p} (64) | ones (64) | v_{2p+1} (64)].  attn@V lhsT for even head
# = [v|ones] -> out rows 0:64 numerator, 64:128 denominator; odd head
# = [ones|v] -> out rows 0:64 denominator, 64:128 numerator.
VB_LT = 1536

_CACHE = {}
TRACE = False


@with_exitstack
def _attention_kernel(ctx: ExitStack, tc: tile.TileContext, aps: dict):
    nc = tc.nc

    sb = ctx.enter_context(tc.tile_pool(name="persist", bufs=1))
    wns_pool = ctx.enter_context(tc.tile_pool(name="wns", bufs=2))
    pt_pool = ctx.enter_context(tc.tile_pool(name="pt", bufs=10))
    stage = ctx.enter_context(tc.tile_pool(name="stage", bufs=2))
    dram = ctx.enter_context(tc.tile_pool(name="dram", bufs=2, space="DRAM"))

    # ---- persistent SBUF tensors ----
    xT = sb.tile([128, 8 * 1024], BF)      # [d-part, dt*1024 + l]
    wq = sb.tile([128, 8 * 1024], BF)      # [d-part, dt*1024 + e]
    wk = sb.tile([128, 8 * 1024], BF)
    wv = sb.tile([128, 8 * 1024], BF)
    wo = sb.tile([128, 8 * 1024], BF)      # [e-part, et*1024 + e']
    qT = sb.tile([128, 8 * 1024], BF)      # [e-part, et*1024 + l]
    # kz: per-head K tile [128 e-rows, h*1024 + k]; the head's 64 e-dims sit
    # at their parity rows, the other 64 rows are ZERO.  Score matmuls then
    # contract over K=128 (full PE width, ~1.7x faster than K=64) with
    # rhs = qT full 128 rows (the zero lhsT rows kill the other head's q).
    kz = sb.tile([128, 16 * 1024], BF)
    vb = sb.tile([128, 8 * VB_LT], BF)     # [l-part, lt*1536 + pair*192 + ...]
    oT = sb.tile([128, 8 * 1024], BF)      # [e-part, et*1024 + l]
    xtails = sb.tile([128, 64], BF)        # [d-part, dt*8 + bb]
    sel = sb.tile([64, 8], BF)             # one-hot row selector (per-core data)
    tri = sb.tile([128, 128], BF)          # causal 0/1 mask: 1 if p<=f else 0
    mbias = sb.tile([128, 8], F32)         # key-padding additive bias per k-block
    nsb = sb.tile([8, 3072], BF)           # my position's q|k|v for all batches
    fullg = sb.tile([64, 3072], BF)        # gathered: row n*8+bb

    # ---- input DMAs (gpsimd software DGE; consolidated) ----
    def chunked(ap):
        return ap.rearrange("(dt p) c -> p dt c", p=128)

    def chunked_dst(t, n=8):
        return t[:].rearrange("p (dt c) -> p dt c", dt=n)

    nc.gpsimd.dma_start(chunked_dst(xT)[:, 0:4, :], chunked(aps["xT"])[:, 0:4, :])
    nc.gpsimd.dma_start(chunked_dst(xT)[:, 4:8, :], chunked(aps["xT"])[:, 4:8, :])
    for et in range(8):
        nc.gpsimd.dma_start(
            chunked_dst(wk)[:, :, et * 128: et * 128 + 128],
            chunked(aps["wkT"])[:, :, et * 128: et * 128 + 128],
        )
    nc.gpsimd.dma_start(xtails[:], aps["xtails"][:])
    nc.gpsimd.dma_start(tri[:], aps["tri"][:])
    nc.gpsimd.dma_start(mbias[:], aps["maskbias"][:])
    nc.gpsimd.dma_start(sel[:], aps["sel"][:])

    # zero halves of kz: even heads use rows 0:64 (zero 64:128), odd heads
    # use rows 64:128 (zero 0:64)
    kz4 = kz[:].rearrange("p (pr two c) -> p pr two c", pr=8, two=2)
    nc.vector.memset(kz4[64:128, :, 0:1, :], 0.0)
    nc.vector.memset(kz4[0:64, :, 1:2, :], 0.0)

    # ones columns of vb (one strided memset per lt; vector engine is idle now)
    for lt in range(8):
        blk = vb[:, lt * VB_LT: (lt + 1) * VB_LT]
        nc.vector.memset(
            blk.rearrange("p (pr x) -> p pr x", pr=8)[:, :, 64:128], 1.0
        )

    def proj_qk(w, dst):
        for et in range(8):
            for lg in range(2):
                acc = ps.tile([128, 512], F32, name="sp", bufs=5)
                for dt in range(8):
                    nc.tensor.matmul(
                        acc[:],
                        w[:, dt * 1024 + et * 128: dt * 1024 + et * 128 + 128],
                        xT[:, dt * 1024 + lg * 512: dt * 1024 + lg * 512 + 512],
                        start=(dt == 0),
                        stop=(dt == 7),
                    )
                if dst is None:  # K projection -> kz parity blocks
                    c0 = 2 * et * 1024 + lg * 512
                    c1 = (2 * et + 1) * 1024 + lg * 512
                    nc.vector.tensor_copy(kz[0:64, c0: c0 + 512], acc[0:64, :])
                    nc.vector.tensor_copy(kz[64:128, c1: c1 + 512], acc[64:128, :])
                else:
                    nc.vector.tensor_copy(
                        dst[:, et * 1024 + lg * 512: et * 1024 + lg * 512 + 512],
                        acc[:],
                    )

    # ---- phase A+B: K projection interleaved with ns projections ----
    # ns step s (= half*8 + dt) is injected after K-group s so the AllGather
    # input is ready ~as the K projection finishes (maximizes skew tolerance)
    with tc.tile_pool(name="psA", bufs=4, space="PSUM") as ps:
        with tc.tile_pool(name="psNS", bufs=1, space="PSUM") as nsps:
            pp = None
            for et in range(8):
                for lg in range(2):
                    acc = ps.tile([128, 512], F32, name="sp", bufs=5)
                    for dt in range(8):
                        nc.tensor.matmul(
                            acc[:],
                            wk[:, dt * 1024 + et * 128: dt * 1024 + et * 128 + 128],
                            xT[:, dt * 1024 + lg * 512: dt * 1024 + lg * 512 + 512],
                            start=(dt == 0),
                            stop=(dt == 7),
                        )
                    c0 = 2 * et * 1024 + lg * 512
                    c1 = (2 * et + 1) * 1024 + lg * 512
                    nc.vector.tensor_copy(kz[0:64, c0: c0 + 512], acc[0:64, :])
                    nc.vector.tensor_copy(kz[64:128, c1: c1 + 512], acc[64:128, :])

                    # one ns step
                    s = et * 2 + lg
                    half, dt = s // 8, s % 8
                    if dt == 0:
                        pp = nsps.tile([8, 1536], F32, name="pp")
                    wt = wns_pool.tile([128, 1536], BF)
                    nc.gpsimd.dma_start(
                        wt[:],
                        aps["wnsT"][
                            dt * 128: dt * 128 + 128,
                            half * 1536: half * 1536 + 1536,
                        ],
                    )
                    for ck in range(3):
                        nc.tensor.matmul(
                            pp[:, ck * 512: ck * 512 + 512],
                            xtails[:, bass.ts(dt, 8)],
                            wt[:, bass.ts(ck, 512)],
                            start=(dt == 0),
                            stop=(dt == 7),
                        )
                    if dt == 7:
                        nc.vector.tensor_copy(
                            nsb[:, half * 1536: half * 1536 + 1536], pp[:]
                        )

        gin = dram.tile([8, 3072], BF)
        gout = dram.tile([64, 3072], BF)
        nc.gpsimd.dma_start(gin[:], nsb[:])
        nc.gpsimd.collective_compute(
            "AllGather",
            mybir.AluOpType.bypass,
            replica_groups=[list(range(NCORES))],
            ins=[gin.opt()],
            outs=[gout.opt()],
        )

        # remaining weights (issued after the collective so the gather is not
        # delayed; transfers overlap the Q/V projections)
        nc.gpsimd.dma_start(chunked_dst(wq), chunked(aps["wqT"]))
        nc.gpsimd.dma_start(chunked_dst(wv), chunked(aps["wvT"]))
        nc.gpsimd.dma_start(chunked_dst(wo), chunked(aps["woutT"]))
        nc.gpsimd.dma_start(fullg[:], gout[:])

        # ---- phase C: Q projection ----
        proj_qk(wq, qT)

        # ---- phase D: V projection, vb layout with shared ones blocks ----
        def vproj_group(lt, eg):
            acc = ps.tile([128, 512], F32, name="sp", bufs=5)
            for dt in range(8):
                nc.tensor.matmul(
                    acc[:],
                    xT[:, dt * 1024 + lt * 128: dt * 1024 + lt * 128 + 128],
                    wv[:, dt * 1024 + eg * 512: dt * 1024 + eg * 512 + 512],
                    start=(dt == 0),
                    stop=(dt == 7),
                )
            # heads eg*8+hh -> pair (eg*4 + hh//2), parity hh%2
            dst = vb[:, lt * VB_LT + eg * 768: lt * VB_LT + eg * 768 + 768]
            dst3 = dst.rearrange("p (pr x) -> p pr x", pr=4)
            src3 = acc[:, :].rearrange("p (pr x) -> p pr x", pr=4)
            nc.vector.tensor_copy(dst3[:, :, 0:64], src3[:, :, 0:64])
            nc.vector.tensor_copy(dst3[:, :, 128:192], src3[:, :, 64:128])

        for lt in range(8):
            for eg in range(2):
                vproj_group(lt, eg)

        # ---- attention ----
        use_bias = aps["use_bias"]

        def attn_scores(h, g):
            et, r0 = h // 2, (h % 2) * 64
            nj = 4 * g + 4
            qbase = et * 1024 + g * 512
            pts = []
            for j in range(nj):
                lead = max(0, j * 128 - g * 512)
                sp = ps.tile([128, 512], F32, name="sp", bufs=5)
                nc.tensor.matmul(
                    sp[:, lead:512],
                    kz[:, h * 1024 + j * 128: h * 1024 + j * 128 + 128],
                    qT[:, qbase + lead: qbase + 512],
                    start=True,
                    stop=True,
                )
                pt = pt_pool.tile([128, 512], BF)
                kw = {"bias": mbias[:, j: j + 1]} if use_bias else {}
                nc.scalar.activation(
                    pt[:, lead:512],
                    sp[:, lead:512],
                    mybir.ActivationFunctionType.Exp,
                    scale=0.125,
                    **kw,
                )
                if j >= 4 * g:  # leading q-sub-block is the diagonal block
                    nc.vector.tensor_tensor(
                        pt[:, lead: lead + 128],
                        pt[:, lead: lead + 128],
                        tri[:],
                        mybir.AluOpType.mult,
                    )
                pts.append((pt, lead))
            return (h, g, pts)

        def attn_av(state, psa):
            h, g, pts = state
            et, r0 = h // 2, (h % 2) * 64
            nj = 4 * g + 4
            qbase = et * 1024 + g * 512
            op = psa.tile([128, 512], F32)
            for j, (pt, lead) in enumerate(pts):
                vcol = j * VB_LT + (h // 2) * 192 + (h % 2) * 64
                nc.tensor.matmul(
                    op[:, lead:512],
                    vb[:, vcol: vcol + 128],
                    pt[:, lead:512],
                    start=(j == 0),
                    stop=(j == nj - 1),
                    skip_group_check=(j > 0),
                )
            nb = 64 * (h % 2)   # numerator base partition
            db = 64 - nb        # denominator base partition
            rb = stage.tile([64, 512], F32, name="rb")
            if db == 0:
                nc.vector.reciprocal_approx_fast(rb[:], op[0:64, :])
            else:
                # reciprocal_approx_fast requires base partition 0: stage den
                dcp = stage.tile([64, 512], F32, name="dcp")
                nc.vector.tensor_copy(dcp[:], op[64:128, :])
                nc.vector.reciprocal_approx_fast(rb[:], dcp[:])
            nc.vector.tensor_tensor(
                oT[r0:r0 + 64, qbase: qbase + 512],
                op[nb:nb + 64, :],
                rb[:],
                mybir.AluOpType.mult,
            )

        def outproj_group(lt, eg):
            acc = ps.tile([128, 512], F32, name="sp", bufs=5)
            for et in range(8):
                nc.tensor.matmul(
                    acc[:],
                    oT[:, et * 1024 + lt * 128: et * 1024 + lt * 128 + 128],
                    wo[:, et * 1024 + eg * 512: et * 1024 + eg * 512 + 512],
                    start=(et == 0),
                    stop=(et == 7),
                )
            ys = stage.tile([128, 512], BF, name="ys")
            nc.vector.tensor_copy(ys[:], acc[:])
            nc.gpsimd.dma_start(
                aps["y"][lt * 128: lt * 128 + 128, eg * 512: eg * 512 + 512],
                ys[:],
            )

        def attn_phase(g, psa, filler=None):
            # software-pipelined by one group: scores(i+1) issue before
            # attnV(i), so the exp chain has a full group of slack.
            # `filler` groups (out-proj) are injected between attention
            # groups to fill tensor idle while the scalar engine crunches exp.
            prev = None
            for h in range(H):
                st = attn_scores(h, g)
                if prev is not None:
                    attn_av(prev, psa)
                if filler and h % 2 == 1:
                    filler.pop(0)()
                prev = st
            attn_av(prev, psa)

        with tc.tile_pool(name="psB", bufs=3, space="PSUM") as psa:
            # g=0 q-groups have no dependence on the gathered tails
            attn_phase(0, psa)

            # out-proj lt0-3 only needs g=0 oT columns; running it here delays
            # the gather-dependent work, adding AllGather skew tolerance
            for lt in range(4):
                for eg in range(2):
                    outproj_group(lt, eg)

            # ---- tails: overwrite q/k rows 1016..1023, v rows from gather ----
            for c2 in range(16):
                tp = ps.tile([128, 512], F32, name="sp", bufs=5)
                nc.tensor.matmul(
                    tp[:, 0:8],
                    fullg[:, c2 * 128: c2 * 128 + 128],
                    sel[:],
                    start=True,
                    stop=True,
                )
                et = c2 % 8
                Copy = mybir.ActivationFunctionType.Copy
                if c2 < 8:
                    nc.scalar.activation(
                        qT[:, et * 1024 + 1016: et * 1024 + 1024], tp[:, 0:8],
                        Copy, scale=1.0,
                    )
                else:
                    ce = 2 * et * 1024 + 1016
                    co = (2 * et + 1) * 1024 + 1016
                    nc.scalar.activation(
                        kz[0:64, ce: ce + 8], tp[0:64, 0:8], Copy, scale=1.0
                    )
                    nc.scalar.activation(
                        kz[64:128, co: co + 8], tp[64:128, 0:8], Copy, scale=1.0
                    )
            for vg in range(2):
                tp = ps.tile([128, 512], F32, name="sp", bufs=5)
                nc.tensor.matmul(
                    tp[0:8, :],
                    sel[:],
                    fullg[:, 2048 + vg * 512: 2048 + vg * 512 + 512],
                    start=True,
                    stop=True,
                )
                vt = stage.tile([8, 512], BF, name="vt", bufs=1)
                nc.vector.tensor_copy(vt[:], tp[0:8, :])
                # scatter into vb tail partitions 120..127, lt=7 slots
                base = 7 * VB_LT + vg * 768
                dst3 = vb[120:128, base: base + 768].rearrange(
                    "p (pr x) -> p pr x", pr=4
                )
                src3 = vt[:, :].rearrange("p (pr x) -> p pr x", pr=4)
                nc.gpsimd.dma_start(dst3[:, :, 0:64], src3[:, :, 0:64])
                nc.gpsimd.dma_start(dst3[:, :, 128:192], src3[:, :, 64:128])

            # ---- attention: g=1 q-groups ----
            attn_phase(1, psa)

            # ---- output projection tail: lt4-7 ----
            for lt in range(4, 8):
                for eg in range(2):
                    outproj_group(lt, eg)


def _build(use_bias=True):
    key = ("nc", use_bias)
    if key in _CACHE:
        return _CACHE[key]
    nc = bacc.Bacc("TRN2", target_bir_lowering=False, debug=False, num_devices=NCORES)
    aps = {}
    for name, shape, dt in [
        ("xT", [1024, 1024], BF),
        ("wqT", [1024, 1024], BF),
        ("wkT", [1024, 1024], BF),
        ("wvT", [1024, 1024], BF),
        ("woutT", [1024, 1024], BF),
        ("wnsT", [1024, 3072], BF),
        ("xtails", [128, 64], BF),
        ("sel", [64, 8], BF),
        ("tri", [128, 128], BF),
        ("maskbias", [128, 8], F32),
    ]:
        aps[name] = nc.dram_tensor(name, shape, dt, kind="ExternalInput").ap()
    aps["y"] = nc.dram_tensor("y", [1024, 1024], BF, kind="ExternalOutput").ap()
    aps["use_bias"] = use_bias

    with tile.TileContext(nc) as tc:
        _attention_kernel(tc, aps)
    nc.compile()
    _CACHE[key] = nc
    return nc


def kernel(x, key_padding_mask, Wq_s, Wk_s, Wv_s, Wq_ns, Wk_ns, Wv_ns, W_out, **kw):
    x = np.asarray(x, np.float32)
    mask = np.asarray(key_padding_mask)
    bf = ml_dtypes.bfloat16

    wqT = np.ascontiguousarray(np.asarray(Wq_s, np.float32).T.astype(bf))
    wkT = np.ascontiguousarray(np.asarray(Wk_s, np.float32).T.astype(bf))
    wvT = np.ascontiguousarray(np.asarray(Wv_s, np.float32).T.astype(bf))
    woT = np.ascontiguousarray(np.asarray(W_out, np.float32).T.astype(bf))
    tri = np.where(
        np.arange(128)[:, None] <= np.arange(128)[None, :], 1.0, 0.0
    ).astype(bf)

    Wq_ns = np.asarray(Wq_ns, np.float32)
    Wk_ns = np.asarray(Wk_ns, np.float32)
    Wv_ns = np.asarray(Wv_ns, np.float32)

    in_maps = []
    for c in range(NCORES):
        xT = np.ascontiguousarray(x[c].T.astype(bf))
        # [128 p, dt*8+bb] -- exact SBUF layout, contiguous DMA
        xtails = np.ascontiguousarray(
            x[:, LS + c, :].T.reshape(8, 128, 8).transpose(1, 0, 2).reshape(128, 64)
        ).astype(bf)
        wnsT = np.ascontiguousarray(
            np.concatenate([Wq_ns[c].T, Wk_ns[c].T, Wv_ns[c].T], axis=1).astype(bf)
        )
        selm = np.zeros((64, 8), bf)
        for n in range(NS):
            selm[n * 8 + c, n] = 1.0
        mb = np.where(mask[c], 0.0, NEG).astype(np.float32).reshape(8, 128).T
        mb = np.ascontiguousarray(mb)
        in_maps.append(
            {
                "xT": xT,
                "wqT": wqT,
                "wkT": wkT,
                "wvT": wvT,
                "woutT": woT,
                "wnsT": wnsT,
                "xtails": xtails,
                "sel": selm,
                "tri": tri,
                "maskbias": mb,
            }
        )

    nc = _build(use_bias=not bool(mask.all()))
    res = run_bass_kernel_spmd(nc, in_maps, list(range(NCORES)), trace=TRACE)
    _CACHE["exec_time_ns"] = res.exec_time_ns
    _CACHE["res"] = res
    out = np.stack(
        [np.asarray(res.results[c]["y"], dtype=np.float32) for c in range(NCORES)],
        axis=0,
    )
    return out


# revision 27
# speedup vs baseline: 1.1097x; 1.1097x over previous
"""Mixed causal attention (B=8,L=1024,D=1024,H=16,NS=8) on 8 TRN2 cores.

Sharding: data-parallel over batch (core b owns batch b) for projections,
attention, out-proj.  The per-position (ns) projection weights are sharded
by position: core c computes q/k/v for position 1016+c for ALL batches
(reads only Wq_ns[c],Wk_ns[c],Wv_ns[c]), then an AllGather distributes the
3x[8,1024] results; each core extracts its batch's 8 tail rows with a
one-hot selection matmul (the program is SPMD-identical, so per-core row
selection is driven by a per-core input, not program structure).

Attention layout: scores [k-part, q-free]; V tile carries a shared ones
column block per head pair so attn@V emits the numerator and a 64-way
replicated softmax denominator in one matmul; normalization is then one
fast approximate reciprocal + one multiply on the vector engine.
All score/exp/attn@V work is causally range-trimmed per k-block.
"""

import sys
import os
from contextlib import ExitStack

import numpy as np

sys.path.insert(0, "/opt/trn_rl_repo")

import ml_dtypes  # noqa: E402
import concourse.bass as bass  # noqa: E402
import concourse.tile as tile  # noqa: E402
from concourse import bacc, mybir  # noqa: E402
from concourse._compat import with_exitstack  # noqa: E402
from concourse.bass_utils import run_bass_kernel_spmd  # noqa: E402

B, L, D, H, NS = 8, 1024, 1024, 16, 8
HD = D // H          # 64
LS = L - NS          # 1016
NCORES = 8
NEG = -1.0e9
BF = mybir.dt.bfloat16
F32 = mybir.dt.float32

# vb layout: per lt block of 1536 cols, 8 head-pairs of 192 cols each:
# [v_{2p} (64) | ones (64) | v_{2p+1} (64)].  attn@V lhsT for even head
# = [v|ones] -> out rows 0:64 numerator, 64:128 denominator; odd head
# = [ones|v] -> out rows 0:64 denominator, 64:128 numerator.
VB_LT = 1536

_CACHE = {}
TRACE = False


@with_exitstack
def _attention_kernel(ctx: ExitStack, tc: tile.TileContext, aps: dict):
    nc = tc.nc

    sb = ctx.enter_context(tc.tile_pool(name="persist", bufs=1))
    wns_pool = ctx.enter_context(tc.tile_pool(name="wns", bufs=2))
    pt_pool = ctx.enter_context(tc.tile_pool(name="pt", bufs=8))
    stage = ctx.enter_context(tc.tile_pool(name="stage", bufs=2))
    dram = ctx.enter_context(tc.tile_pool(name="dram", bufs=2, space="DRAM"))

    # ---- persistent SBUF tensors ----
    xT = sb.tile([128, 8 * 1024], BF)      # [d-part, dt*1024 + l]
    wq = sb.tile([128, 8 * 1024], BF)      # [d-part, dt*1024 + e]
    wk = sb.tile([128, 8 * 1024], BF)
    wv = sb.tile([128, 8 * 1024], BF)
    wo = sb.tile([128, 8 * 1024], BF)      # [e-part, et*1024 + e']
    qT = sb.tile([128, 8 * 1024], BF)      # [e-part, et*1024 + l]
    # kz: per-head K tile [128 e-rows, h*1024 + k]; the head's 64 e-dims sit
    # at their parity rows, the other 64 rows are ZERO.  Score matmuls then
    # contract over K=128 (full PE width, ~1.7x faster than K=64) with
    # rhs = qT full 128 rows (the zero lhsT rows kill the other head's q).
    kz = sb.tile([128, 16 * 1024], BF)
    vb = sb.tile([128, 8 * VB_LT], BF)     # [l-part, lt*1536 + pair*192 + ...]
    oT = sb.tile([128, 8 * 1024], BF)      # [e-part, et*1024 + l]
    xtails = sb.tile([128, 64], BF)        # [d-part, dt*8 + bb]
    sel = sb.tile([64, 8], BF)             # one-hot row selector (per-core data)
    tri = sb.tile([128, 128], BF)          # causal 0/1 mask: 1 if p<=f else 0
    mbias = sb.tile([128, 8], F32)         # key-padding additive bias per k-block
    nsb = sb.tile([8, 3072], BF)           # my position's q|k|v for all batches
    fullg = sb.tile([64, 3072], BF)        # gathered: row n*8+bb

    # ---- input DMAs (gpsimd software DGE; consolidated) ----
    def chunked(ap):
        return ap.rearrange("(dt p) c -> p dt c", p=128)

    def chunked_dst(t, n=8):
        return t[:].rearrange("p (dt c) -> p dt c", dt=n)

    nc.gpsimd.dma_start(chunked_dst(xT)[:, 0:4, :], chunked(aps["xT"])[:, 0:4, :])
    nc.gpsimd.dma_start(chunked_dst(xT)[:, 4:8, :], chunked(aps["xT"])[:, 4:8, :])
    for et in range(8):
        nc.gpsimd.dma_start(
            chunked_dst(wk)[:, :, et * 128: et * 128 + 128],
            chunked(aps["wkT"])[:, :, et * 128: et * 128 + 128],
        )
    nc.gpsimd.dma_start(xtails[:], aps["xtails"][:])
    nc.gpsimd.dma_start(tri[:], aps["tri"][:])
    nc.gpsimd.dma_start(mbias[:], aps["maskbias"][:])
    nc.gpsimd.dma_start(sel[:], aps["sel"][:])

    # zero halves of kz: even heads use rows 0:64 (zero 64:128), odd heads
    # use rows 64:128 (zero 0:64)
    kz4 = kz[:].rearrange("p (pr two c) -> p pr two c", pr=8, two=2)
    nc.vector.memset(kz4[64:128, :, 0:1, :], 0.0)
    nc.vector.memset(kz4[0:64, :, 1:2, :], 0.0)

    # ones columns of vb (one strided memset per lt; vector engine is idle now)
    for lt in range(8):
        blk = vb[:, lt * VB_LT: (lt + 1) * VB_LT]
        nc.vector.memset(
            blk.rearrange("p (pr x) -> p pr x", pr=8)[:, :, 64:128], 1.0
        )

    def proj_qk(w, dst):
        for et in range(8):
            for lg in range(2):
                acc = ps.tile([128, 512], F32, name="sp", bufs=5)
                for dt in range(8):
                    nc.tensor.matmul(
                        acc[:],
                        w[:, dt * 1024 + et * 128: dt * 1024 + et * 128 + 128],
                        xT[:, dt * 1024 + lg * 512: dt * 1024 + lg * 512 + 512],
                        start=(dt == 0),
                        stop=(dt == 7),
                    )
                if dst is None:  # K projection -> kz parity blocks
                    c0 = 2 * et * 1024 + lg * 512
                    c1 = (2 * et + 1) * 1024 + lg * 512
                    nc.vector.tensor_copy(kz[0:64, c0: c0 + 512], acc[0:64, :])
                    nc.vector.tensor_copy(kz[64:128, c1: c1 + 512], acc[64:128, :])
                else:
                    nc.vector.tensor_copy(
                        dst[:, et * 1024 + lg * 512: et * 1024 + lg * 512 + 512],
                        acc[:],
                    )

    # ---- phase A+B: K projection interleaved with ns projections ----
    # ns step s (= half*8 + dt) is injected after K-group s so the AllGather
    # input is ready ~as the K projection finishes (maximizes skew tolerance)
    with tc.tile_pool(name="psA", bufs=4, space="PSUM") as ps:
        with tc.tile_pool(name="psNS", bufs=1, space="PSUM") as nsps:
            pp = None
            for et in range(8):
                for lg in range(2):
                    acc = ps.tile([128, 512], F32, name="sp", bufs=5)
                    for dt in range(8):
                        nc.tensor.matmul(
                            acc[:],
                            wk[:, dt * 1024 + et * 128: dt * 1024 + et * 128 + 128],
                            xT[:, dt * 1024 + lg * 512: dt * 1024 + lg * 512 + 512],
                            start=(dt == 0),
                            stop=(dt == 7),
                        )
                    c0 = 2 * et * 1024 + lg * 512
                    c1 = (2 * et + 1) * 1024 + lg * 512
                    nc.vector.tensor_copy(kz[0:64, c0: c0 + 512], acc[0:64, :])
                    nc.vector.tensor_copy(kz[64:128, c1: c1 + 512], acc[64:128, :])

                    # one ns step
                    s = et * 2 + lg
                    half, dt = s // 8, s % 8
                    if dt == 0:
                        pp = nsps.tile([8, 1536], F32, name="pp")
                    wt = wns_pool.tile([128, 1536], BF)
                    nc.gpsimd.dma_start(
                        wt[:],
                        aps["wnsT"][
                            dt * 128: dt * 128 + 128,
                            half * 1536: half * 1536 + 1536,
                        ],
                    )
                    for ck in range(3):
                        nc.tensor.matmul(
                            pp[:, ck * 512: ck * 512 + 512],
                            xtails[:, bass.ts(dt, 8)],
                            wt[:, bass.ts(ck, 512)],
                            start=(dt == 0),
                            stop=(dt == 7),
                        )
                    if dt == 7:
                        nc.vector.tensor_copy(
                            nsb[:, half * 1536: half * 1536 + 1536], pp[:]
                        )

        gin = dram.tile([8, 3072], BF)
        gout = dram.tile([64, 3072], BF)
        nc.gpsimd.dma_start(gin[:], nsb[:])
        nc.gpsimd.collective_compute(
            "AllGather",
            mybir.AluOpType.bypass,
            replica_groups=[list(range(NCORES))],
            ins=[gin.opt()],
            outs=[gout.opt()],
        )

        # remaining weights (issued after the collective so the gather is not
        # delayed; transfers overlap the Q/V projections)
        nc.gpsimd.dma_start(chunked_dst(wq), chunked(aps["wqT"]))
        nc.gpsimd.dma_start(chunked_dst(wv), chunked(aps["wvT"]))
        nc.gpsimd.dma_start(chunked_dst(wo), chunked(aps["woutT"]))
        nc.gpsimd.dma_start(fullg[:], gout[:])

        # ---- phase C: Q projection ----
        proj_qk(wq, qT)

        # ---- phase D: V projection, vb layout with shared ones blocks ----
        def vproj_group(lt, eg):
            acc = ps.tile([128, 512], F32, name="sp", bufs=5)
            for dt in range(8):
                nc.tensor.matmul(
                    acc[:],
                    xT[:, dt * 1024 + lt * 128: dt * 1024 + lt * 128 + 128],
                    wv[:, dt * 1024 + eg * 512: dt * 1024 + eg * 512 + 512],
                    start=(dt == 0),
                    stop=(dt == 7),
                )
            # heads eg*8+hh -> pair (eg*4 + hh//2), parity hh%2
            dst = vb[:, lt * VB_LT + eg * 768: lt * VB_LT + eg * 768 + 768]
            dst3 = dst.rearrange("p (pr x) -> p pr x", pr=4)
            src3 = acc[:, :].rearrange("p (pr x) -> p pr x", pr=4)
            nc.vector.tensor_copy(dst3[:, :, 0:64], src3[:, :, 0:64])
            nc.vector.tensor_copy(dst3[:, :, 128:192], src3[:, :, 64:128])

        for lt in range(8):
            for eg in range(2):
                vproj_group(lt, eg)

        # ---- attention ----
        use_bias = aps["use_bias"]

        def attn_scores(h, g):
            et, r0 = h // 2, (h % 2) * 64
            nj = 4 * g + 4
            qbase = et * 1024 + g * 512
            pts = []
            for j in range(nj):
                lead = max(0, j * 128 - g * 512)
                sp = ps.tile([128, 512], F32, name="sp", bufs=5)
                nc.tensor.matmul(
                    sp[:, lead:512],
                    kz[:, h * 1024 + j * 128: h * 1024 + j * 128 + 128],
                    qT[:, qbase + lead: qbase + 512],
                    start=True,
                    stop=True,
                )
                pt = pt_pool.tile([128, 512], BF)
                kw = {"bias": mbias[:, j: j + 1]} if use_bias else {}
                nc.scalar.activation(
                    pt[:, lead:512],
                    sp[:, lead:512],
                    mybir.ActivationFunctionType.Exp,
                    scale=0.125,
                    **kw,
                )
                if j >= 4 * g:  # leading q-sub-block is the diagonal block
                    nc.vector.tensor_tensor(
                        pt[:, lead: lead + 128],
                        pt[:, lead: lead + 128],
                        tri[:],
                        mybir.AluOpType.mult,
                    )
                pts.append((pt, lead))
            return (h, g, pts)

        def attn_av(state, psa):
            h, g, pts = state
            et, r0 = h // 2, (h % 2) * 64
            nj = 4 * g + 4
            qbase = et * 1024 + g * 512
            op = psa.tile([128, 512], F32)
            for j, (pt, lead) in enumerate(pts):
                vcol = j * VB_LT + (h // 2) * 192 + (h % 2) * 64
                nc.tensor.matmul(
                    op[:, lead:512],
                    vb[:, vcol: vcol + 128],
                    pt[:, lead:512],
                    start=(j == 0),
                    stop=(j == nj - 1),
                    skip_group_check=(j > 0),
                )
            nb = 64 * (h % 2)   # numerator base partition
            db = 64 - nb        # denominator base partition
            rb = stage.tile([64, 512], F32, name="rb")
            if db == 0:
                nc.vector.reciprocal_approx_fast(rb[:], op[0:64, :])
            else:
                # reciprocal_approx_fast requires base partition 0: stage den
                dcp = stage.tile([64, 512], F32, name="dcp")
                nc.vector.tensor_copy(dcp[:], op[64:128, :])
                nc.vector.reciprocal_approx_fast(rb[:], dcp[:])
            nc.vector.tensor_tensor(
                oT[r0:r0 + 64, qbase: qbase + 512],
                op[nb:nb + 64, :],
                rb[:],
                mybir.AluOpType.mult,
            )

        def outproj_group(lt, eg):
            acc = ps.tile([128, 512], F32, name="sp", bufs=5)
            for et in range(8):
                nc.tensor.matmul(
                    acc[:],
                    oT[:, et * 1024 + lt * 128: et * 1024 + lt * 128 + 128],
                    wo[:, et * 1024 + eg * 512: et * 1024 + eg * 512 + 512],
                    start=(et == 0),
                    stop=(et == 7),
                )
            ys = stage.tile([128, 512], BF, name="ys")
            nc.vector.tensor_copy(ys[:], acc[:])
            nc.gpsimd.dma_start(
                aps["y"][lt * 128: lt * 128 + 128, eg * 512: eg * 512 + 512],
                ys[:],
            )

        def attn_phase(g, psa, filler=None):
            # software-pipelined by one group: scores(i+1) issue before
            # attnV(i), so the exp chain has a full group of slack.
            # `filler` groups (out-proj) are injected between attention
            # groups to fill tensor idle while the scalar engine crunches exp.
            prev = None
            for h in range(H):
                st = attn_scores(h, g)
                if prev is not None:
                    attn_av(prev, psa)
                if filler and h % 2 == 1:
                    filler.pop(0)()
                prev = st
            attn_av(prev, psa)

        with tc.tile_pool(name="psB", bufs=3, space="PSUM") as psa:
            # g=0 q-groups have no dependence on the gathered tails
            attn_phase(0, psa)

            # out-proj lt0-3 only needs g=0 oT columns; running it here delays
            # the gather-dependent work, adding AllGather skew tolerance
            for lt in range(4):
                for eg in range(2):
                    outproj_group(lt, eg)

            # ---- tails: overwrite q/k rows 1016..1023, v rows from gather ----
            for c2 in range(16):
                tp = ps.tile([128, 512], F32, name="sp", bufs=5)
                nc.tensor.matmul(
                    tp[:, 0:8],
                    fullg[:, c2 * 128: c2 * 128 + 128],
                    sel[:],
                    start=True,
                    stop=True,
                )
                et = c2 % 8
                Copy = mybir.ActivationFunctionType.Copy
                if c2 < 8:
                    nc.scalar.activation(
                        qT[:, et * 1024 + 1016: et * 1024 + 1024], tp[:, 0:8],
                        Copy, scale=1.0,
                    )
                else:
                    ce = 2 * et * 1024 + 1016
                    co = (2 * et + 1) * 1024 + 1016
                    nc.scalar.activation(
                        kz[0:64, ce: ce + 8], tp[0:64, 0:8], Copy, scale=1.0
                    )
                    nc.scalar.activation(
                        kz[64:128, co: co + 8], tp[64:128, 0:8], Copy, scale=1.0
                    )
            for vg in range(2):
                tp = ps.tile([128, 512], F32, name="sp", bufs=5)
                nc.tensor.matmul(
                    tp[0:8, :],
                    sel[:],
                    fullg[:, 2048 + vg * 512: 2048 + vg * 512 + 512],
                    start=True,
                    stop=True,
                )
                vt = stage.tile([8, 512], BF, name="vt", bufs=1)
                nc.vector.tensor_copy(vt[:], tp[0:8, :])
                # scatter into vb tail partitions 120..127, lt=7 slots
                base = 7 * VB_LT + vg * 768
                dst3 = vb[120:128, base: base + 768].rearrange(
                    "p (pr x) -> p pr x", pr=4
                )
                src3 = vt[:, :].rearrange("p (pr x) -> p pr x", pr=4)
                nc.gpsimd.dma_start(dst3[:, :, 0:64], src3[:, :, 0:64])
                nc.gpsimd.dma_start(dst3[:, :, 128:192], src3[:, :, 64:128])

            # ---- attention: g=1 q-groups ----
            attn_phase(1, psa)

            # ---- output projection tail: lt4-7 ----
            for lt in range(4, 8):
                for eg in range(2):
                    outproj_group(lt, eg)


def _build(use_bias=True):
    key = ("nc", use_bias)
    if key in _CACHE:
        return _CACHE[key]
    nc = bacc.Bacc("TRN2", target_bir_lowering=False, debug=False, num_devices=NCORES)
    aps = {}
    for name, shape, dt in [
        ("xT", [1024, 1024], BF),
        ("wqT", [1024, 1024], BF),
        ("wkT", [1024, 1024], BF),
        ("wvT", [1024, 1024], BF),
        ("woutT", [1024, 1024], BF),
        ("wnsT", [1024, 3072], BF),
        ("xtails", [128, 64], BF),
        ("sel", [64, 8], BF),
        ("tri", [128, 128], BF),
        ("maskbias", [128, 8], F32),
    ]:
        aps[name] = nc.dram_tensor(name, shape, dt, kind="ExternalInput").ap()
    aps["y"] = nc.dram_tensor("y", [1024, 1024], BF, kind="ExternalOutput").ap()
    aps["use_bias"] = use_bias

    with tile.TileContext(nc) as tc:
        _attention_kernel(tc, aps)
    nc.compile()
    _CACHE[key] = nc
    return nc


def kernel(x, key_padding_mask, Wq_s, Wk_s, Wv_s, Wq_ns, Wk_ns, Wv_ns, W_out, **kw):
    x = np.asarray(x, np.float32)
    mask = np.asarray(key_padding_mask)
    bf = ml_dtypes.bfloat16

    wqT = np.ascontiguousarray(np.asarray(Wq_s, np.float32).T.astype(bf))
    wkT = np.ascontiguousarray(np.asarray(Wk_s, np.float32).T.astype(bf))
    wvT = np.ascontiguousarray(np.asarray(Wv_s, np.float32).T.astype(bf))
    woT = np.ascontiguousarray(np.asarray(W_out, np.float32).T.astype(bf))
    tri = np.where(
        np.arange(128)[:, None] <= np.arange(128)[None, :], 1.0, 0.0
    ).astype(bf)

    Wq_ns = np.asarray(Wq_ns, np.float32)
    Wk_ns = np.asarray(Wk_ns, np.float32)
    Wv_ns = np.asarray(Wv_ns, np.float32)

    in_maps = []
    for c in range(NCORES):
        xT = np.ascontiguousarray(x[c].T.astype(bf))
        # [128 p, dt*8+bb] -- exact SBUF layout, contiguous DMA
        xtails = np.ascontiguousarray(
            x[:, LS + c, :].T.reshape(8, 128, 8).transpose(1, 0, 2).reshape(128, 64)
        ).astype(bf)
        wnsT = np.ascontiguousarray(
            np.concatenate([Wq_ns[c].T, Wk_ns[c].T, Wv_ns[c].T], axis=1).astype(bf)
        )
        selm = np.zeros((64, 8), bf)
        for n in range(NS):
            selm[n * 8 + c, n] = 1.0
        mb = np.where(mask[c], 0.0, NEG).astype(np.float32).reshape(8, 128).T
        mb = np.ascontiguousarray(mb)
        in_maps.append(
            {
                "xT": xT,
                "wqT": wqT,
                "wkT": wkT,
                "wvT": wvT,
                "woutT": woT,
                "wnsT": wnsT,
                "xtails": xtails,
                "sel": selm,
                "tri": tri,
                "maskbias": mb,
            }
        )

    nc = _build(use_bias=not bool(mask.all()))
    res = run_bass_kernel_spmd(nc, in_maps, list(range(NCORES)), trace=TRACE)
    _CACHE["exec_time_ns"] = res.exec_time_ns
    _CACHE["res"] = res
    out = np.stack(
        [np.asarray(res.results[c]["y"], dtype=np.float32) for c in range(NCORES)],
        axis=0,
    )
    return out
